# revision 7
# baseline (speedup 1.0000x reference)
"""Trainium2 Bass kernel for nn_AttentionBlock (b=32, c=32, L=511, G=32, H=1).

Sharding: data-parallel over batch (4 batches/core on 8 cores) for
norm/qkv/attention; in-kernel AllGather of the attention output; the
self-convolution (out_ch = b mixes batches) is computed per-core for the
core's 4 batches with the full gathered tensor as the conv kernel.

Self-conv as matmuls: y[n,o,l] = sum_{i,w} U[n,i,l+w-255] * U[o,i,w].
w is blocked as w = 4*w0 + g; contraction K = (g,i) = 128 partitions.
  lhsT: UW[(g,i), 32*w0+o] = U[o,i,4*w0+g]            (weights, shared)
  rhs:  R_j[(g,i), f]      = Upad[n_j, i, f+g-255]    (per-batch, shifted)
giving, per (w0, j): psum[o, l] += sum_{g,i} UW[.,o] * R_j[., 4*w0+l].
The 4 local batches run concurrently via PE column-group tiling.
"""

import os
import numpy as np

import concourse.bass as bass
import concourse.bacc as bacc
import concourse.tile as tile
import concourse.mybir as mybir
import concourse.bass_utils as bass_utils

N_CORES = 8
B, C, L = 32, 32, 511
B_LOC = B // N_CORES  # 4
EPS = 1e-5
SCALE = float(C) ** -0.5

F32 = mybir.dt.float32
# dtype used for matmul operands ("float32" | "float32r"); same bit layout,
# so tiles stay float32 and APs are bitcast at the matmul call sites.
CONV_DT = getattr(mybir.dt, os.environ.get("KERNEL_CONV_DT", "float32"))
ATTN_DT = getattr(mybir.dt, os.environ.get("KERNEL_ATTN_DT", "float32"))

M_CHUNKS = [(0, 128), (128, 128), (256, 128), (384, 127)]  # m-chunks of L=511
NW0 = 128  # w0 steps; w = 4*w0 + g covers 0..511 (w=511 zero-padded)


def _cast(ap, dt):
    return ap if dt == F32 else ap.bitcast(dt)


def _build_nc():
    nc = bacc.Bacc("TRN2", target_bir_lowering=False, debug=False,
                   num_devices=N_CORES)

    x_in = nc.dram_tensor("x", [B_LOC, C, L], F32, kind="ExternalInput")
    nw_in = nc.dram_tensor("norm_w", [C], F32, kind="ExternalInput")
    nb_in = nc.dram_tensor("norm_b", [C], F32, kind="ExternalInput")
    qkvw_in = nc.dram_tensor("qkv_w", [3 * C, C], F32, kind="ExternalInput")
    qkvb_in = nc.dram_tensor("qkv_b", [3 * C], F32, kind="ExternalInput")
    pw_in = nc.dram_tensor("proj_w", [C, C], F32, kind="ExternalInput")
    pb_in = nc.dram_tensor("proj_b", [C], F32, kind="ExternalInput")
    y_out = nc.dram_tensor("y", [B_LOC, C, L], F32, kind="ExternalOutput")

    with tile.TileContext(nc) as tc:
        with tc.tile_pool(name="const", bufs=1) as cpool, \
             tc.tile_pool(name="work", bufs=2) as wpool, \
             tc.tile_pool(name="esb", bufs=8) as epool, \
             tc.tile_pool(name="big", bufs=1) as bpool, \
             tc.tile_pool(name="psq", bufs=1, space="PSUM") as psq, \
             tc.tile_pool(name="psT", bufs=2, space="PSUM") as psT, \
             tc.tile_pool(name="pvt", bufs=1, space="PSUM") as pvt, \
             tc.tile_pool(name="pout", bufs=1, space="PSUM") as pout, \
             tc.tile_pool(name="pconv", bufs=1, space="PSUM") as pconv, \
             tc.tile_pool(name="dram", bufs=1, space="DRAM") as dram:

            # ---------------- constants / weights ----------------
            x_all = cpool.tile([128, L], F32, tag="x_all")
            nc.sync.dma_start(x_all[:], x_in.ap().rearrange("j c l -> (j c) l"))

            wq_aug = cpool.tile([33, C], F32, tag="wq")
            wk_aug = cpool.tile([33, C], F32, tag="wk")
            wv_aug = cpool.tile([33, C], F32, tag="wv")
            qkvw_ap = qkvw_in.ap()
            qkvb_ap = qkvb_in.ap().rearrange("(p o) -> p o", p=1)
            for t, sl in ((wq_aug, slice(0, 32)), (wk_aug, slice(32, 64)),
                          (wv_aug, slice(64, 96))):
                nc.sync.dma_start(t[0:32, :], qkvw_ap[sl, :].rearrange("o c -> c o"))
                nc.sync.dma_start(t[32:33, :], qkvb_ap[:, sl])

            wp_t = cpool.tile([C, C], F32, tag="wp")
            nc.sync.dma_start(wp_t[:], pw_in.ap().rearrange("o c -> c o"))

            nw_rep = cpool.tile([128, 1], F32, tag="nw")
            nb_rep = cpool.tile([128, 1], F32, tag="nb")
            pb_rep = cpool.tile([128, 1], F32, tag="pb")
            for j in range(B_LOC):
                nc.sync.dma_start(nw_rep[32 * j:32 * j + 32, :],
                                  nw_in.ap().rearrange("(c o) -> c o", o=1))
                nc.sync.dma_start(nb_rep[32 * j:32 * j + 32, :],
                                  nb_in.ap().rearrange("(c o) -> c o", o=1))
                nc.sync.dma_start(pb_rep[32 * j:32 * j + 32, :],
                                  pb_in.ap().rearrange("(c o) -> c o", o=1))

            bc_ones = cpool.tile([33, 32], F32, tag="bc_ones")
            nc.gpsimd.memset(bc_ones[:], 1.0)

            # ---------------- group norm (per-channel over L) ----------------
            bnst = wpool.tile([128, 6], F32, tag="bnst")
            bnag = wpool.tile([128, 2], F32, tag="bnag")
            nc.vector.bn_stats(bnst[:], x_all[:])
            nc.vector.bn_aggr(bnag[:], bnst[:])
            eps_t = cpool.tile([128, 1], F32, tag="eps")
            nc.gpsimd.memset(eps_t[:], EPS)
            std = wpool.tile([128, 1], F32, tag="std")
            nc.scalar.activation(std[:], bnag[:, 1:2],
                                 mybir.ActivationFunctionType.Sqrt,
                                 bias=eps_t[:, 0:1])
            rstd = wpool.tile([128, 1], F32, tag="rstd")
            nc.vector.reciprocal(rstd[:], std[:])
            alpha = wpool.tile([128, 1], F32, tag="alpha")
            beta = wpool.tile([128, 1], F32, tag="beta")
            nc.vector.tensor_mul(alpha[:], rstd[:], nw_rep[:])
            nc.vector.tensor_mul(beta[:], bnag[:, 0:1], alpha[:])
            nc.vector.tensor_sub(beta[:], nb_rep[:], beta[:])
            h_all = cpool.tile([128, L], F32, tag="h_all")
            nc.vector.tensor_scalar(h_all[:], x_all[:], alpha[:, 0:1],
                                    beta[:, 0:1], mybir.AluOpType.mult,
                                    mybir.AluOpType.add)

            # ---------------- attention per local batch ----------------
            cc_in = dram.tile([B_LOC * C * L], F32)
            cc_out = dram.tile([B * C * L], F32, addr_space="Shared")
            u_sb = []
            r_tiles = []
            for j in range(B_LOC):
                hones = wpool.tile([33, L], F32, tag="hones")
                nc.sync.dma_start(hones[0:32, :], h_all[32 * j:32 * j + 32, :])
                nc.gpsimd.memset(hones[32:33, :], 1.0)

                ps_q = psq.tile([32, L], F32, tag="ps_q")
                ps_k = psq.tile([32, L], F32, tag="ps_k")
                nc.tensor.matmul(ps_q[:], _cast(wq_aug[:], ATTN_DT),
                                 _cast(hones[:], ATTN_DT), start=True, stop=True)
                nc.tensor.matmul(ps_k[:], _cast(wk_aug[:], ATTN_DT),
                                 _cast(hones[:], ATTN_DT), start=True, stop=True)
                q_sb = wpool.tile([32, L], F32, tag="q_sb")
                k_sb = wpool.tile([32, L], F32, tag="k_sb")
                nc.vector.tensor_copy(q_sb[:], ps_q[:])
                nc.vector.tensor_copy(k_sb[:], ps_k[:])

                vt_ones = []
                for (m0, mc) in M_CHUNKS:
                    ps_vt = pvt.tile([128, 32], F32, tag="ps_vt")
                    nc.tensor.matmul(ps_vt[0:mc, :],
                                     _cast(hones[:, m0:m0 + mc], ATTN_DT),
                                     _cast(wv_aug[:], ATTN_DT),
                                     start=True, stop=True)
                    vt = epool.tile([128, 33], F32, tag="vt")
                    nc.gpsimd.memset(vt[:, 32:33], 1.0)
                    nc.vector.tensor_copy(vt[0:mc, 0:32], ps_vt[0:mc, :])
                    vt_ones.append(vt)

                esb = []
                for (m0, mc) in M_CHUNKS:
                    ps_sT = psT.tile([128, L], F32, tag="ps_sT")
                    nc.tensor.matmul(ps_sT[0:mc, :],
                                     _cast(k_sb[:, m0:m0 + mc], ATTN_DT),
                                     _cast(q_sb[:], ATTN_DT),
                                     start=True, stop=True)
                    e_t = epool.tile([128, L], F32, tag="e_t")
                    nc.scalar.activation(e_t[0:mc, :], ps_sT[0:mc, :],
                                         mybir.ActivationFunctionType.Exp,
                                         scale=SCALE)
                    esb.append(e_t)

                ps_out = pout.tile([33, L], F32, tag="ps_out")
                for ci, (m0, mc) in enumerate(M_CHUNKS):
                    nc.tensor.matmul(ps_out[:],
                                     _cast(vt_ones[ci][0:mc, :], ATTN_DT),
                                     _cast(esb[ci][0:mc, :], ATTN_DT),
                                     start=(ci == 0),
                                     stop=(ci == len(M_CHUNKS) - 1))

                recip = wpool.tile([33, L], F32, tag="recip")
                nc.vector.reciprocal(recip[32:33, :], ps_out[32:33, :])
                ps_bc = pvt.tile([32, L], F32, tag="ps_bc")
                nc.tensor.matmul(ps_bc[:], _cast(bc_ones[32:33, :], ATTN_DT),
                                 _cast(recip[32:33, :], ATTN_DT),
                                 start=True, stop=True)
                u_un = wpool.tile([32, L], F32, tag="u_un")
                nc.scalar.copy(u_un[:], ps_out[0:32, :])
                u = cpool.tile([32, L], F32, tag=f"u{j}")
                nc.vector.tensor_mul(u[:], u_un[:], ps_bc[:])
                u_sb.append(u)

                nc.sync.dma_start(
                    cc_in[j * C * L:(j + 1) * C * L].rearrange(
                        "(i l) -> i l", i=C), u[:])

                # conv rhs tile for this batch (shifted copies, zero-padded)
                r_t = bpool.tile([128, 1024], F32, tag=f"r{j}")
                nc.gpsimd.memset(r_t[:], 0.0)
                for g in range(4):
                    nc.sync.dma_start(
                        r_t[32 * g:32 * g + 32, 255 - g:255 - g + L], u[:])
                r_tiles.append(r_t)

            # ---------------- all-gather ----------------
            nc.gpsimd.collective_compute(
                "AllGather", mybir.AluOpType.bypass,
                replica_groups=[list(range(N_CORES))],
                ins=[cc_in[:]], outs=[cc_out[:]])

            # ---------------- conv weight tile ----------------
            # UT4[32g+i, 512*o + t] = U[o, i, t+g]  (zero when t+g >= 511),
            # so the w0-lhsT is UT4[:, 4*w0::512] (count 32) with element
            # (g*32+i, o) = U[o, i, 4*w0+g].
            S2 = 512
            ut4 = bpool.tile([128, 32 * S2], F32, tag="ut4")
            cc3 = cc_out[:].rearrange("(o i l) -> o i l", o=B, i=C)
            for g in range(4):
                src = cc3[:, :, g:L].rearrange("o i l -> i o l")
                dst = ut4[32 * g:32 * g + 32, :].rearrange(
                    "p (o t) -> p o t", t=S2)[:, :, 0:L - g]
                nc.sync.dma_start(dst, src)
                pad = ut4[32 * g:32 * g + 32, :].rearrange(
                    "p (o t) -> p o t", t=S2)[:, :, L - g:S2]
                nc.gpsimd.memset(pad, 0.0)

            # ---------------- self-conv ----------------
            ps_conv = pconv.tile([128, L], F32, tag="ps_conv")
            for w0 in range(NW0):
                lhsT = ut4[:, 4 * w0:4 * w0 + 31 * S2 + 1:S2]
                for j in range(B_LOC):
                    nc.tensor.matmul(
                        ps_conv[32 * j:32 * j + 32, :],
                        _cast(lhsT, CONV_DT),
                        _cast(r_tiles[j][:, 4 * w0:4 * w0 + L], CONV_DT),
                        start=(w0 == 0), stop=(w0 == NW0 - 1),
                        tile_position=(0, 32 * j))

            # ---------------- proj + residual ----------------
            ps_p = psT.tile([128, L], F32, tag="ps_sT")
            for j in range(B_LOC):
                ysb = wpool.tile([32, L], F32, tag="ysb")
                nc.vector.tensor_copy(ysb[:], ps_conv[32 * j:32 * j + 32, :])
                nc.tensor.matmul(ps_p[32 * j:32 * j + 32, :],
                                 _cast(wp_t[:], CONV_DT),
                                 _cast(ysb[:], CONV_DT),
                                 start=True, stop=True,
                                 tile_position=(0, 32 * j))
            res = cpool.tile([128, L], F32, tag="res")
            nc.vector.tensor_add(res[:], ps_p[:], x_all[:])
            nc.vector.tensor_scalar_add(res[:], res[:], pb_rep[:, 0:1])
            nc.sync.dma_start(y_out.ap().rearrange("j c l -> (j c) l"), res[:])

    nc.compile()
    return nc


_NC_CACHE = None


def kernel(x, norm_w, norm_b, qkv_w, qkv_b, proj_w, proj_b):
    global _NC_CACHE
    if _NC_CACHE is None:
        _NC_CACHE = _build_nc()
    nc = _NC_CACHE

    x = np.ascontiguousarray(np.asarray(x, dtype=np.float32))
    in_maps = []
    for c in range(N_CORES):
        in_maps.append({
            "x": x[c * B_LOC:(c + 1) * B_LOC],
            "norm_w": np.asarray(norm_w, np.float32),
            "norm_b": np.asarray(norm_b, np.float32),
            "qkv_w": np.asarray(qkv_w, np.float32),
            "qkv_b": np.asarray(qkv_b, np.float32),
            "proj_w": np.asarray(proj_w, np.float32),
            "proj_b": np.asarray(proj_b, np.float32),
        })

    trace = bool(int(os.environ.get("KERNEL_TRACE", "0")))
    res = bass_utils.run_bass_kernel_spmd(
        nc, in_maps, core_ids=list(range(N_CORES)), trace=trace)
    if trace and res.exec_time_ns is not None:
        print(f"HW exec time: {res.exec_time_ns} ns")

    out = np.concatenate([res.results[c]["y"] for c in range(N_CORES)], axis=0)
    return out.astype(np.float32)


# revision 10
# speedup vs baseline: 1.8609x; 1.8609x over previous
"""Trainium2 Bass kernel for nn_AttentionBlock (b=32, c=32, L=511, G=32, H=1).

Sharding: data-parallel over batch (4 batches/core on 8 cores) for
norm/qkv/attention; in-kernel AllGather of the attention output; the
self-convolution (out_ch = b mixes batches) is computed per-core for the
core's 4 batches with the full gathered tensor as the conv kernel.

Self-conv as matmuls: y[n,o,l] = sum_{i,w} U[n,i,l+w-255] * U[o,i,w].
w is blocked as w = 4*w0 + g; contraction K = (g,i) = 128 partitions.
  lhsT: UW[(g,i), 32*w0+o] = U[o,i,4*w0+g]            (weights, shared)
  rhs:  R_j[(g,i), f]      = Upad[n_j, i, f+g-255]    (per-batch, shifted)
giving, per (w0, j): psum[o, l] += sum_{g,i} UW[.,o] * R_j[., 4*w0+l].
The 4 local batches run concurrently via PE column-group tiling.
"""

import os
import numpy as np

import concourse.bass as bass
import concourse.bacc as bacc
import concourse.tile as tile
import concourse.mybir as mybir
import concourse.bass_utils as bass_utils

N_CORES = 8
B, C, L = 32, 32, 511
B_LOC = B // N_CORES  # 4
EPS = 1e-5
SCALE = float(C) ** -0.5

F32 = mybir.dt.float32
# dtype used for matmul operands ("float32" | "float32r"); same bit layout,
# so tiles stay float32 and APs are bitcast at the matmul call sites.
CONV_DT = getattr(mybir.dt, os.environ.get("KERNEL_CONV_DT", "float32"))
ATTN_DT = getattr(mybir.dt, os.environ.get("KERNEL_ATTN_DT", "float32"))

M_CHUNKS = [(0, 128), (128, 128), (256, 128), (384, 127)]  # m-chunks of L=511
NW0 = 128  # w0 steps; w = 4*w0 + g covers 0..511 (w=511 zero-padded)


def _build_nc():
    nc = bacc.Bacc("TRN2", target_bir_lowering=False, debug=False,
                   num_devices=N_CORES)

    x_in = nc.dram_tensor("x", [B_LOC, C, L], F32, kind="ExternalInput")
    nw_in = nc.dram_tensor("norm_w", [C], F32, kind="ExternalInput")
    nb_in = nc.dram_tensor("norm_b", [C], F32, kind="ExternalInput")
    qkvw_in = nc.dram_tensor("qkv_w", [3 * C, C], F32, kind="ExternalInput")
    qkvb_in = nc.dram_tensor("qkv_b", [3 * C], F32, kind="ExternalInput")
    pw_in = nc.dram_tensor("proj_w", [C, C], F32, kind="ExternalInput")
    pb_in = nc.dram_tensor("proj_b", [C], F32, kind="ExternalInput")
    y_out = nc.dram_tensor("y", [B_LOC, C, L], F32, kind="ExternalOutput")

    with tile.TileContext(nc) as tc, \
         nc.allow_low_precision(reason="float32r matmul operands; psum accumulation stays fp32"):
        with tc.tile_pool(name="const", bufs=1) as cpool, \
             tc.tile_pool(name="work", bufs=2) as wpool, \
             tc.tile_pool(name="esb", bufs=8) as epool, \
             tc.tile_pool(name="big", bufs=1) as bpool, \
             tc.tile_pool(name="psq", bufs=1, space="PSUM") as psq, \
             tc.tile_pool(name="psT", bufs=2, space="PSUM") as psT, \
             tc.tile_pool(name="pvt", bufs=1, space="PSUM") as pvt, \
             tc.tile_pool(name="pout", bufs=1, space="PSUM") as pout, \
             tc.tile_pool(name="pconv", bufs=1, space="PSUM") as pconv, \
             tc.tile_pool(name="dram", bufs=1, space="DRAM") as dram:

            # ---------------- constants / weights ----------------
            x_all = cpool.tile([128, L], F32, tag="x_all")
            nc.sync.dma_start(x_all[:], x_in.ap().rearrange("j c l -> (j c) l"))

            wqkv_stage = cpool.tile([33, 3 * C], F32, tag="wqkv_stage")
            qkvw_ap = qkvw_in.ap()
            qkvb_ap = qkvb_in.ap().rearrange("(p o) -> p o", p=1)
            nc.sync.dma_start(wqkv_stage[0:32, :],
                              qkvw_ap.rearrange("o c -> c o"))
            nc.sync.dma_start(wqkv_stage[32:33, :], qkvb_ap[:, :])
            wq_aug = cpool.tile([33, C], ATTN_DT, tag="wq")
            wk_aug = cpool.tile([33, C], ATTN_DT, tag="wk")
            wv_aug = cpool.tile([33, C], ATTN_DT, tag="wv")
            nc.vector.tensor_copy(wq_aug[:], wqkv_stage[:, 0:32])
            nc.vector.tensor_copy(wk_aug[:], wqkv_stage[:, 32:64])
            nc.vector.tensor_copy(wv_aug[:], wqkv_stage[:, 64:96])

            wp_stage = cpool.tile([C, C], F32, tag="wp_stage")
            nc.sync.dma_start(wp_stage[:], pw_in.ap().rearrange("o c -> c o"))
            wp_t = cpool.tile([C, C], CONV_DT, tag="wp")
            nc.vector.tensor_copy(wp_t[:], wp_stage[:])

            nw_rep = cpool.tile([128, 1], F32, tag="nw")
            nb_rep = cpool.tile([128, 1], F32, tag="nb")
            pb_rep = cpool.tile([128, 1], F32, tag="pb")
            for j in range(B_LOC):
                nc.sync.dma_start(nw_rep[32 * j:32 * j + 32, :],
                                  nw_in.ap().rearrange("(c o) -> c o", o=1))
                nc.sync.dma_start(nb_rep[32 * j:32 * j + 32, :],
                                  nb_in.ap().rearrange("(c o) -> c o", o=1))
                nc.sync.dma_start(pb_rep[32 * j:32 * j + 32, :],
                                  pb_in.ap().rearrange("(c o) -> c o", o=1))

            bc_ones = cpool.tile([33, 32], ATTN_DT, tag="bc_ones")
            nc.vector.memset(bc_ones[:], 1.0)

            # ---------------- group norm (per-channel over L) ----------------
            bnst = wpool.tile([128, 6], F32, tag="bnst")
            bnag = wpool.tile([128, 2], F32, tag="bnag")
            nc.vector.bn_stats(bnst[:], x_all[:])
            nc.vector.bn_aggr(bnag[:], bnst[:])
            eps_t = cpool.tile([128, 1], F32, tag="eps")
            nc.gpsimd.memset(eps_t[:], EPS)
            std = wpool.tile([128, 1], F32, tag="std")
            nc.scalar.activation(std[:], bnag[:, 1:2],
                                 mybir.ActivationFunctionType.Sqrt,
                                 bias=eps_t[:, 0:1])
            rstd = wpool.tile([128, 1], F32, tag="rstd")
            nc.vector.reciprocal(rstd[:], std[:])
            alpha = wpool.tile([128, 1], F32, tag="alpha")
            beta = wpool.tile([128, 1], F32, tag="beta")
            nc.vector.tensor_mul(alpha[:], rstd[:], nw_rep[:])
            nc.vector.tensor_mul(beta[:], bnag[:, 0:1], alpha[:])
            nc.vector.tensor_sub(beta[:], nb_rep[:], beta[:])
            h_all = cpool.tile([128, L], ATTN_DT, tag="h_all")
            nc.vector.tensor_scalar(h_all[:], x_all[:], alpha[:, 0:1],
                                    beta[:, 0:1], mybir.AluOpType.mult,
                                    mybir.AluOpType.add)

            # ---------------- attention per local batch ----------------
            cc_in = dram.tile([B_LOC * C * L], CONV_DT)
            cc_out = dram.tile([B * C * L], CONV_DT, addr_space="Shared")
            u_sb = []
            r_tiles = []
            for j in range(B_LOC):
                hones = wpool.tile([33, L], ATTN_DT, tag="hones")
                nc.sync.dma_start(hones[0:32, :], h_all[32 * j:32 * j + 32, :])
                nc.vector.memset(hones[32:33, :], 1.0)

                ps_q = psq.tile([32, L], F32, tag="ps_q")
                ps_k = psq.tile([32, L], F32, tag="ps_k")
                nc.tensor.matmul(ps_q[:], wq_aug[:],
                                 hones[:], start=True, stop=True)
                nc.tensor.matmul(ps_k[:], wk_aug[:],
                                 hones[:], start=True, stop=True)
                q_sb = wpool.tile([32, L], ATTN_DT, tag="q_sb")
                k_sb = wpool.tile([32, L], ATTN_DT, tag="k_sb")
                nc.vector.tensor_copy(q_sb[:], ps_q[:])
                nc.vector.tensor_copy(k_sb[:], ps_k[:])

                vt_ones = []
                for (m0, mc) in M_CHUNKS:
                    ps_vt = pvt.tile([128, 32], F32, tag="ps_vt")
                    nc.tensor.matmul(ps_vt[0:mc, :],
                                     hones[:, m0:m0 + mc],
                                     wv_aug[:],
                                     start=True, stop=True)
                    vt = epool.tile([128, 33], ATTN_DT, tag="vt")
                    nc.vector.memset(vt[:, 32:33], 1.0)
                    nc.vector.tensor_copy(vt[0:mc, 0:32], ps_vt[0:mc, :])
                    vt_ones.append(vt)

                esb = []
                for (m0, mc) in M_CHUNKS:
                    ps_sT = psT.tile([128, L], F32, tag="ps_sT")
                    nc.tensor.matmul(ps_sT[0:mc, :],
                                     k_sb[:, m0:m0 + mc],
                                     q_sb[:],
                                     start=True, stop=True)
                    e_t = epool.tile([128, L], ATTN_DT, tag="e_t")
                    nc.scalar.activation(e_t[0:mc, :], ps_sT[0:mc, :],
                                         mybir.ActivationFunctionType.Exp,
                                         scale=SCALE)
                    esb.append(e_t)

                ps_out = pout.tile([33, L], F32, tag="ps_out")
                for ci, (m0, mc) in enumerate(M_CHUNKS):
                    nc.tensor.matmul(ps_out[:],
                                     vt_ones[ci][0:mc, :],
                                     esb[ci][0:mc, :],
                                     start=(ci == 0),
                                     stop=(ci == len(M_CHUNKS) - 1))

                recip = wpool.tile([33, L], ATTN_DT, tag="recip")
                nc.vector.reciprocal(recip[32:33, :], ps_out[32:33, :])
                ps_bc = pvt.tile([32, L], F32, tag="ps_bc")
                nc.tensor.matmul(ps_bc[:], bc_ones[32:33, :],
                                 recip[32:33, :],
                                 start=True, stop=True)
                u_un = wpool.tile([32, L], F32, tag="u_un")
                nc.scalar.copy(u_un[:], ps_out[0:32, :])
                u = cpool.tile([32, L], CONV_DT, tag=f"u{j}")
                nc.vector.tensor_mul(u[:], u_un[:], ps_bc[:])
                u_sb.append(u)

                nc.sync.dma_start(
                    cc_in[j * C * L:(j + 1) * C * L].rearrange(
                        "(i l) -> i l", i=C), u[:])

                # conv rhs tile for this batch (shifted copies, zero-padded)
                r_t = bpool.tile([128, 1024], CONV_DT, tag=f"r{j}")
                nc.vector.memset(r_t[:], 0.0)
                for g in range(4):
                    nc.sync.dma_start(
                        r_t[32 * g:32 * g + 32, 255 - g:255 - g + L], u[:])
                r_tiles.append(r_t)

            # ---------------- all-gather ----------------
            nc.gpsimd.collective_compute(
                "AllGather", mybir.AluOpType.bypass,
                replica_groups=[list(range(N_CORES))],
                ins=[cc_in[:]], outs=[cc_out[:]])

            # ---------------- conv weight tile ----------------
            # UT4[32g+i, 512*o + t] = U[o, i, t+g]  (zero when t+g >= 511),
            # so the w0-lhsT is UT4[:, 4*w0::512] (count 32) with element
            # (g*32+i, o) = U[o, i, 4*w0+g].
            S2 = 512
            ut4 = bpool.tile([128, 32 * S2], CONV_DT, tag="ut4")
            cc3 = cc_out[:].rearrange("(o i l) -> o i l", o=B, i=C)
            for g in range(4):
                src = cc3[:, :, g:L].rearrange("o i l -> i o l")
                dst = ut4[32 * g:32 * g + 32, :].rearrange(
                    "p (o t) -> p o t", t=S2)[:, :, 0:L - g]
                nc.sync.dma_start(dst, src)
                pad = ut4[32 * g:32 * g + 32, :].rearrange(
                    "p (o t) -> p o t", t=S2)[:, :, L - g:S2]
                nc.vector.memset(pad, 0.0)

            # ---------------- self-conv ----------------
            ps_conv = pconv.tile([128, L], F32, tag="ps_conv")
            for w0 in range(NW0):
                lhsT = ut4[:, 4 * w0:4 * w0 + 31 * S2 + 1:S2]
                for j in range(B_LOC):
                    nc.tensor.matmul(
                        ps_conv[32 * j:32 * j + 32, :],
                        lhsT,
                        r_tiles[j][:, 4 * w0:4 * w0 + L],
                        start=(w0 == 0), stop=(w0 == NW0 - 1),
                        tile_position=(0, 32 * j))

            # ---------------- proj + residual ----------------
            ps_p = psT.tile([128, L], F32, tag="ps_sT")
            for j in range(B_LOC):
                ysb = wpool.tile([32, L], CONV_DT, tag="ysb")
                nc.vector.tensor_copy(ysb[:], ps_conv[32 * j:32 * j + 32, :])
                nc.tensor.matmul(ps_p[32 * j:32 * j + 32, :],
                                 wp_t[:],
                                 ysb[:],
                                 start=True, stop=True,
                                 tile_position=(0, 32 * j))
            res = cpool.tile([128, L], F32, tag="res")
            nc.vector.tensor_add(res[:], ps_p[:], x_all[:])
            nc.vector.tensor_scalar_add(res[:], res[:], pb_rep[:, 0:1])
            nc.sync.dma_start(y_out.ap().rearrange("j c l -> (j c) l"), res[:])

    nc.compile()
    return nc


_NC_CACHE = None


def kernel(x, norm_w, norm_b, qkv_w, qkv_b, proj_w, proj_b):
    global _NC_CACHE
    if _NC_CACHE is None:
        _NC_CACHE = _build_nc()
    nc = _NC_CACHE

    x = np.ascontiguousarray(np.asarray(x, dtype=np.float32))
    in_maps = []
    for c in range(N_CORES):
        in_maps.append({
            "x": x[c * B_LOC:(c + 1) * B_LOC],
            "norm_w": np.asarray(norm_w, np.float32),
            "norm_b": np.asarray(norm_b, np.float32),
            "qkv_w": np.asarray(qkv_w, np.float32),
            "qkv_b": np.asarray(qkv_b, np.float32),
            "proj_w": np.asarray(proj_w, np.float32),
            "proj_b": np.asarray(proj_b, np.float32),
        })

    trace = bool(int(os.environ.get("KERNEL_TRACE", "0")))
    res = bass_utils.run_bass_kernel_spmd(
        nc, in_maps, core_ids=list(range(N_CORES)), trace=trace)
    if trace and res.exec_time_ns is not None:
        print(f"HW exec time: {res.exec_time_ns} ns")

    out = np.concatenate([res.results[c]["y"] for c in range(N_CORES)], axis=0)
    return out.astype(np.float32)


# revision 15
# speedup vs baseline: 1.9244x; 1.0341x over previous
"""Trainium2 Bass kernel for nn_AttentionBlock (b=32, c=32, L=511, G=32, H=1).

Sharding: data-parallel over batch (4 batches/core on 8 cores) for
norm/qkv/attention; in-kernel AllGather (split in two l-chunks so the
first half of the self-conv overlaps the second gather); the self-conv
(out_ch = b mixes batches) is computed per-core for the core's 4 batches
with the full gathered tensor as the conv kernel.

Attention is batch-packed on the PE array: the 4 local batches run as
concurrent 32x32 tiles (diagonal tiles for q/k, row tiles for scores,
column tiles for the attention-value product), with softmax implemented
as exp(S^T) on ScalarE, sum via ones-matmuls, and a matmul-broadcast of
the reciprocal.

Self-conv as matmuls: y[n,o,l] = sum_{i,w} U[n,i,l+w-255] * U[o,i,w].
w is blocked as w = 4*w0 + g; contraction K = (g,i) = 128 partitions.
  lhsT: UT4x[(g,i), (r,o',t)] = U[4r+o', i, l0+t+g]   (weights, shared)
  rhs:  R_j[(g,i), f]         = Upad[n_j, i, f+g-255] (per-batch, shifted)
The 4 local batches run concurrently via PE column-group tiling.
"""

import os
import numpy as np

import concourse.bass as bass
import concourse.bacc as bacc
import concourse.tile as tile
import concourse.mybir as mybir
import concourse.bass_utils as bass_utils

N_CORES = 8
B, C, L = 32, 32, 511
B_LOC = B // N_CORES  # 4
EPS = 1e-5
SCALE = float(C) ** -0.5

F32 = mybir.dt.float32
CONV_DT = getattr(mybir.dt, os.environ.get("KERNEL_CONV_DT", "float16"))
ATTN_DT = getattr(mybir.dt, os.environ.get("KERNEL_ATTN_DT", "float16"))

M_CHUNKS = [(0, 128), (128, 128), (256, 128), (384, 127)]  # m-chunks of L=511
# l-chunks of the gathered tensor: chunk X covers conv w0 range, sourcing
# l in [l0, l0+259). 259 = 256 + 3 so that t+g never leaves the chunk
# (reads for l >= 511 land in the zero tail of u4).
CHUNK_A = dict(l0=0, w0s=range(0, 64))
CHUNK_B = dict(l0=256, w0s=range(64, 128))
CW = 259


def _build_nc():
    nc = bacc.Bacc("TRN2", target_bir_lowering=False, debug=False,
                   num_devices=N_CORES)

    x_in = nc.dram_tensor("x", [B_LOC, C, L], F32, kind="ExternalInput")
    nw_in = nc.dram_tensor("norm_w", [C], F32, kind="ExternalInput")
    nb_in = nc.dram_tensor("norm_b", [C], F32, kind="ExternalInput")
    qkvw_in = nc.dram_tensor("qkv_w", [3 * C, C], F32, kind="ExternalInput")
    qkvb_in = nc.dram_tensor("qkv_b", [3 * C], F32, kind="ExternalInput")
    pw_in = nc.dram_tensor("proj_w", [C, C], F32, kind="ExternalInput")
    pb_in = nc.dram_tensor("proj_b", [C], F32, kind="ExternalInput")
    y_out = nc.dram_tensor("y", [B_LOC, C, L], F32, kind="ExternalOutput")

    with tile.TileContext(nc) as tc, \
         nc.allow_low_precision(reason="fp16 matmul operands; psum stays fp32"):
        with tc.tile_pool(name="const", bufs=1) as cpool, \
             tc.tile_pool(name="work", bufs=2) as wpool, \
             tc.tile_pool(name="esb", bufs=8) as epool, \
             tc.tile_pool(name="big", bufs=1) as bpool, \
             tc.tile_pool(name="pqk", bufs=1, space="PSUM") as pqk, \
             tc.tile_pool(name="psT", bufs=2, space="PSUM") as psT, \
             tc.tile_pool(name="pvt", bufs=1, space="PSUM") as pvt, \
             tc.tile_pool(name="pout", bufs=1, space="PSUM") as pout, \
             tc.tile_pool(name="dram", bufs=1, space="DRAM") as dram:

            # ---------------- weights / constants ----------------
            x_all = cpool.tile([128, L], F32, tag="x_all")
            nc.sync.dma_start(x_all[:], x_in.ap().rearrange("j c l -> (j c) l"))

            # qkv_w: load raw, cast to 16-bit, transpose via DMA (16-bit,
            # 128-wide xbar tiles; junk outside the valid block is unread)
            qw_raw = cpool.tile([3 * C, C], F32, tag="qw_raw")
            nc.sync.dma_start(qw_raw[:], qkvw_in.ap())
            qw16 = cpool.tile([128, 128], ATTN_DT, tag="qw16")
            nc.vector.tensor_copy(qw16[0:96, 0:32], qw_raw[:])
            qwT = cpool.tile([128, 128], ATTN_DT, tag="qwT")
            nc.sync.dma_start(qwT[:], qw16[:], transpose=True)
            wqkvT16 = qwT[0:32, 0:96]
            wq_rep = cpool.tile([128, C], ATTN_DT, tag="wq_rep")
            wk_rep = cpool.tile([128, C], ATTN_DT, tag="wk_rep")
            wv_rep = cpool.tile([128, C], ATTN_DT, tag="wv_rep")
            for b_ in range(B_LOC):
                sl = slice(32 * b_, 32 * b_ + 32)
                nc.sync.dma_start(wq_rep[sl, :], wqkvT16[:, 0:32])
                nc.sync.dma_start(wk_rep[sl, :], wqkvT16[:, 32:64])
                nc.sync.dma_start(wv_rep[sl, :], wqkvT16[:, 64:96])

            pw_raw = cpool.tile([C, C], F32, tag="pw_raw")
            nc.sync.dma_start(pw_raw[:], pw_in.ap())
            pw16 = cpool.tile([128, 128], CONV_DT, tag="pw16")
            nc.vector.tensor_copy(pw16[0:32, 0:32], pw_raw[:])
            pwT = cpool.tile([128, 128], CONV_DT, tag="pwT")
            nc.sync.dma_start(pwT[:], pw16[:], transpose=True)
            wp_t = pwT[0:32, 0:32]

            nw_rep = cpool.tile([128, 1], F32, tag="nw")
            nb_rep = cpool.tile([128, 1], F32, tag="nb")
            pb_rep = cpool.tile([128, 1], F32, tag="pb")
            qb_rep = cpool.tile([128, 1], F32, tag="qb")
            kb_rep = cpool.tile([128, 1], F32, tag="kb")
            qkvb_col = qkvb_in.ap().rearrange("(o u) -> o u", u=1)
            for b_ in range(B_LOC):
                sl = slice(32 * b_, 32 * b_ + 32)
                nc.sync.dma_start(nw_rep[sl, :],
                                  nw_in.ap().rearrange("(c o) -> c o", o=1))
                nc.sync.dma_start(nb_rep[sl, :],
                                  nb_in.ap().rearrange("(c o) -> c o", o=1))
                nc.sync.dma_start(pb_rep[sl, :],
                                  pb_in.ap().rearrange("(c o) -> c o", o=1))
                nc.sync.dma_start(qb_rep[sl, :], qkvb_col[0:32, :])
                nc.sync.dma_start(kb_rep[sl, :], qkvb_col[32:64, :])

            # v-bias broadcast [128, 32] via K=1 ones matmul (vb is on free dim)
            vb_st = cpool.tile([1, C], F32, tag="vb_st")
            nc.sync.dma_start(vb_st[:], qkvb_in.ap()
                              .rearrange("(p o) -> p o", p=1)[:, 64:96])
            vb_row = cpool.tile([1, C], ATTN_DT, tag="vb_row")
            nc.vector.tensor_copy(vb_row[:], vb_st[:])
            ones_r0 = cpool.tile([1, 128], ATTN_DT, tag="ones_r0")
            nc.vector.memset(ones_r0[:], 1.0)
            ps_vb = pvt.tile([128, C], F32, tag="ps_vt", name="ps_vb")
            nc.tensor.matmul(ps_vb[:], ones_r0[:], vb_row[:], start=True,
                             stop=True)
            vb_bcast = cpool.tile([128, C], F32, tag="vb_bcast")
            nc.scalar.copy(vb_bcast[:], ps_vb[:])

            bc_ones = cpool.tile([128, 32], ATTN_DT, tag="bc_ones")
            nc.vector.memset(bc_ones[:], 1.0)
            ones128 = cpool.tile([128, 1], ATTN_DT, tag="ones128")
            nc.vector.memset(ones128[:], 1.0)

            # ---------------- group norm (per-channel over L) ----------------
            bnst = wpool.tile([128, 6], F32, tag="bnst")
            bnag = wpool.tile([128, 2], F32, tag="bnag")
            nc.vector.bn_stats(bnst[:], x_all[:])
            nc.vector.bn_aggr(bnag[:], bnst[:])
            eps_t = cpool.tile([128, 1], F32, tag="eps")
            nc.gpsimd.memset(eps_t[:], EPS)
            std = wpool.tile([128, 1], F32, tag="std")
            nc.scalar.activation(std[:], bnag[:, 1:2],
                                 mybir.ActivationFunctionType.Sqrt,
                                 bias=eps_t[:, 0:1])
            rstd = wpool.tile([128, 1], F32, tag="rstd")
            nc.vector.reciprocal(rstd[:], std[:])
            alpha = wpool.tile([128, 1], F32, tag="alpha")
            beta = wpool.tile([128, 1], F32, tag="beta")
            nc.vector.tensor_mul(alpha[:], rstd[:], nw_rep[:])
            nc.vector.tensor_mul(beta[:], bnag[:, 0:1], alpha[:])
            nc.vector.tensor_sub(beta[:], nb_rep[:], beta[:])
            h_all = cpool.tile([128, L], ATTN_DT, tag="h_all")
            nc.vector.tensor_scalar(h_all[:], x_all[:], alpha[:, 0:1],
                                    beta[:, 0:1], mybir.AluOpType.mult,
                                    mybir.AluOpType.add)

            # ---------------- attention, 4 batches packed ----------------
            # q/k: diagonal 32x32 tiles (row group b -> col group b)
            ps_q4 = pqk.tile([128, L], F32, tag="ps_q4")
            ps_k4 = pqk.tile([128, L], F32, tag="ps_k4")
            for b_ in range(B_LOC):
                sl = slice(32 * b_, 32 * b_ + 32)
                nc.tensor.matmul(ps_q4[sl, :], wq_rep[sl, :], h_all[sl, :],
                                 start=True, stop=True,
                                 tile_position=(32 * b_, 32 * b_))
                nc.tensor.matmul(ps_k4[sl, :], wk_rep[sl, :], h_all[sl, :],
                                 start=True, stop=True,
                                 tile_position=(32 * b_, 32 * b_))
            q4_sb = cpool.tile([128, L], ATTN_DT, tag="q4_sb")
            k4_sb = cpool.tile([128, L], ATTN_DT, tag="k4_sb")
            nc.vector.tensor_scalar(q4_sb[:], ps_q4[:], qb_rep[:, 0:1], None,
                                    mybir.AluOpType.add)
            nc.vector.tensor_scalar(k4_sb[:], ps_k4[:], kb_rep[:, 0:1], None,
                                    mybir.AluOpType.add)

            # V^T chunks per batch: [m-chunk, 32] (+ v bias)
            vt_sb = {}
            for ci, (m0, mc) in enumerate(M_CHUNKS):
                for b_ in range(B_LOC):
                    sl = slice(32 * b_, 32 * b_ + 32)
                    ps_vt = pvt.tile([128, C], F32, tag="ps_vt",
                                     name=f"ps_vt{ci}_{b_}")
                    nc.tensor.matmul(ps_vt[0:mc, :], h_all[sl, m0:m0 + mc],
                                     wv_rep[sl, :], start=True, stop=True,
                                     tile_position=(32 * b_, 0))
                    vt = epool.tile([128, C], ATTN_DT, tag="vt",
                                    name=f"vt{ci}_{b_}")
                    nc.vector.tensor_add(vt[0:mc, :], ps_vt[0:mc, :],
                                         vb_bcast[0:mc, :])
                    vt_sb[(ci, b_)] = vt

            # scores S^T = K^T Q chunks, exp via ScalarE (scale folded in)
            e_sb = {}
            for ci, (m0, mc) in enumerate(M_CHUNKS):
                for b_ in range(B_LOC):
                    sl = slice(32 * b_, 32 * b_ + 32)
                    ps_sT = psT.tile([128, L], F32, tag="ps_sT",
                                     name=f"ps_sT{ci}_{b_}")
                    nc.tensor.matmul(ps_sT[0:mc, :], k4_sb[sl, m0:m0 + mc],
                                     q4_sb[sl, :], start=True, stop=True,
                                     tile_position=(32 * b_, 0))
                    e_t = epool.tile([128, L], ATTN_DT, tag="e_t",
                                     name=f"e{ci}_{b_}")
                    nc.scalar.activation(e_t[0:mc, :], ps_sT[0:mc, :],
                                         mybir.ActivationFunctionType.Exp,
                                         scale=SCALE)
                    e_sb[(ci, b_)] = e_t

            # out = V @ E^T (col-group per batch) and sumexp (M=1 per batch)
            ps_o4 = pout.tile([128, L], F32, tag="ps_o4")
            ps_se = pout.tile([128, L], F32, tag="ps_se")
            for ci, (m0, mc) in enumerate(M_CHUNKS):
                st, sp = (ci == 0), (ci == len(M_CHUNKS) - 1)
                for b_ in range(B_LOC):
                    sl = slice(32 * b_, 32 * b_ + 32)
                    nc.tensor.matmul(ps_o4[sl, :], vt_sb[(ci, b_)][0:mc, :],
                                     e_sb[(ci, b_)][0:mc, :], start=st, stop=sp,
                                     tile_position=(0, 32 * b_))
                    nc.tensor.matmul(ps_se[32 * b_:32 * b_ + 1, :],
                                     ones128[0:mc, :], e_sb[(ci, b_)][0:mc, :],
                                     start=st, stop=sp,
                                     tile_position=(0, 32 * b_))

            recip_sb = cpool.tile([128, L], ATTN_DT, tag="recip_sb")
            ps_bc4 = pvt.tile([128, L], F32, tag="ps_bc4")
            for b_ in range(B_LOC):
                r0 = slice(32 * b_, 32 * b_ + 1)
                sl = slice(32 * b_, 32 * b_ + 32)
                nc.vector.reciprocal(recip_sb[r0, :], ps_se[r0, :])
                nc.tensor.matmul(ps_bc4[sl, :], bc_ones[r0, :],
                                 recip_sb[r0, :], start=True, stop=True,
                                 tile_position=(32 * b_, 32 * b_))
            u_un4 = cpool.tile([128, L], F32, tag="u_un4")
            nc.scalar.copy(u_un4[:], ps_o4[:])
            # u4 padded to 520: columns [511, 520) stay zero so shifted
            # conv-weight reads for l >= 511 see zeros.
            u4 = cpool.tile([128, 520], CONV_DT, tag="u4")
            nc.vector.memset(u4[:, 508:520], 0.0)
            nc.vector.tensor_mul(u4[:, 0:L], u_un4[:], ps_bc4[:])

            # ---------------- all-gather (two l-chunks) ----------------
            # shard layout [i, o'(local batch), l-chunk]
            cc_in, cc_out = {}, {}
            for key, ch in (("A", CHUNK_A), ("B", CHUNK_B)):
                cc_in[key] = dram.tile([C * B_LOC * CW], CONV_DT,
                                       name=f"cc_in_{key}")
                cc_out[key] = dram.tile([N_CORES * C * B_LOC * CW], CONV_DT,
                                        addr_space="Shared",
                                        name=f"cc_out_{key}")
                l0 = ch["l0"]
                dst3 = cc_in[key][:].rearrange("(i o l) -> i o l", i=C,
                                               o=B_LOC)
                for b_ in range(B_LOC):
                    nc.sync.dma_start(dst3[:, b_:b_ + 1, :],
                                      u4[32 * b_:32 * b_ + 32, l0:l0 + CW])
                nc.gpsimd.collective_compute(
                    "AllGather", mybir.AluOpType.bypass,
                    replica_groups=[list(range(N_CORES))],
                    ins=[cc_in[key][:]], outs=[cc_out[key][:]])

            # conv rhs tiles (shifted copies of local batches, zero-padded)
            r_tiles = []
            for j in range(B_LOC):
                r_t = bpool.tile([128, 1024], CONV_DT, tag=f"r{j}",
                                 name=f"r{j}")
                nc.vector.memset(r_t[:], 0.0)
                for g in range(4):
                    nc.sync.dma_start(
                        r_t[32 * g:32 * g + 32, 255 - g:255 - g + L],
                        u4[32 * j:32 * j + 32, 0:L])
                r_tiles.append(r_t)

            # conv weights: each g-block is the full [i, (r,o,l)] shard
            # stored at free offset (3-g), so ut4X[(g,i), CW*(4r+o') + t]
            # = U[4r+o', i, l0 + t - 3 + g]; the per-o spill of (3-g)
            # elements lands in the next block's t < 3 region, never read
            # (lhsT slices use t = 4*w0 - l0 + 3 >= 3).
            ut4 = {}
            for key in ("A", "B"):
                t_u = bpool.tile([128, B * CW + 3], CONV_DT, tag=f"ut4{key}",
                                 name=f"ut4{key}")
                srcg = cc_out[key][:].rearrange(
                    "(r i o l) -> i r o l", r=N_CORES, i=C, o=B_LOC)
                for g in range(4):
                    dst = t_u[32 * g:32 * g + 32,
                              (3 - g):(3 - g) + B * CW].rearrange(
                        "p (r o t) -> p r o t", r=N_CORES, o=B_LOC)
                    nc.sync.dma_start(dst, srcg)
                ut4[key] = t_u

            # ---------------- self-conv ----------------
            ps_conv = psT.tile([128, L], F32, tag="ps_sT", name="ps_conv")
            for key, ch in (("A", CHUNK_A), ("B", CHUNK_B)):
                u3 = ut4[key][:, 0:B * CW].rearrange(
                    "p (r o t) -> p r o t", r=N_CORES, o=B_LOC)
                for w0 in ch["w0s"]:
                    toff = 4 * w0 - ch["l0"] + 3
                    lhsT = u3[:, :, :, toff:toff + 1]
                    for j in range(B_LOC):
                        nc.tensor.matmul(
                            ps_conv[32 * j:32 * j + 32, :],
                            lhsT,
                            r_tiles[j][:, 4 * w0:4 * w0 + L],
                            start=(w0 == 0), stop=(w0 == 127),
                            tile_position=(0, 32 * j))

            # ---------------- proj + residual ----------------
            ps_p = psT.tile([128, L], F32, tag="ps_sT", name="ps_p")
            for j in range(B_LOC):
                ysb = wpool.tile([32, L], CONV_DT, tag="ysb", name=f"ysb{j}")
                nc.vector.tensor_copy(ysb[:], ps_conv[32 * j:32 * j + 32, :])
                nc.tensor.matmul(ps_p[32 * j:32 * j + 32, :], wp_t[:], ysb[:],
                                 start=True, stop=True,
                                 tile_position=(0, 32 * j))
            res = cpool.tile([128, L], F32, tag="res")
            nc.vector.tensor_add(res[:], ps_p[:], x_all[:])
            nc.vector.tensor_scalar_add(res[:], res[:], pb_rep[:, 0:1])
            nc.sync.dma_start(y_out.ap().rearrange("j c l -> (j c) l"), res[:])

    nc.compile()
    return nc


_NC_CACHE = None


def kernel(x, norm_w, norm_b, qkv_w, qkv_b, proj_w, proj_b):
    global _NC_CACHE
    if _NC_CACHE is None:
        _NC_CACHE = _build_nc()
    nc = _NC_CACHE

    x = np.ascontiguousarray(np.asarray(x, dtype=np.float32))
    in_maps = []
    for c in range(N_CORES):
        in_maps.append({
            "x": x[c * B_LOC:(c + 1) * B_LOC],
            "norm_w": np.asarray(norm_w, np.float32),
            "norm_b": np.asarray(norm_b, np.float32),
            "qkv_w": np.asarray(qkv_w, np.float32),
            "qkv_b": np.asarray(qkv_b, np.float32),
            "proj_w": np.asarray(proj_w, np.float32),
            "proj_b": np.asarray(proj_b, np.float32),
        })

    trace = bool(int(os.environ.get("KERNEL_TRACE", "0")))
    res = bass_utils.run_bass_kernel_spmd(
        nc, in_maps, core_ids=list(range(N_CORES)), trace=trace)
    if trace and res.exec_time_ns is not None:
        print(f"HW exec time: {res.exec_time_ns} ns")

    out = np.concatenate([res.results[c]["y"] for c in range(N_CORES)], axis=0)
    return out.astype(np.float32)


# revision 29
# speedup vs baseline: 2.2929x; 1.1915x over previous
"""Trainium2 Bass kernel for nn_AttentionBlock (b=32, c=32, L=511, G=32, H=1).

Sharding: data-parallel over batch (4 batches/core on 8 cores) for
norm/qkv/attention; in-kernel AllGather (split in two l-chunks so the
first half of the self-conv overlaps the second gather); the self-conv
(out_ch = b mixes batches) is computed per-core for the core's 4 batches
with the full gathered tensor as the conv kernel.

Attention is batch-packed on the PE array: the 4 local batches run as
concurrent 32x32 tiles (diagonal tiles for q/k, row tiles for scores,
column tiles for the attention-value product), with softmax implemented
as exp(S^T) on ScalarE, sum via ones-matmuls, and a matmul-broadcast of
the reciprocal.

Self-conv as matmuls: y[n,o,l] = sum_{i,w} U[n,i,l+w-255] * U[o,i,w].
w is blocked as w = 4*w0 + g; contraction K = (g,i) = 128 partitions.
  lhsT: UT4x[(g,i), (r,o',t)] = U[4r+o', i, l0+t+g]   (weights, shared)
  rhs:  R_j[(g,i), f]         = Upad[n_j, i, f+g-255] (per-batch, shifted)
The 4 local batches run concurrently via PE column-group tiling.
"""

import os
import numpy as np

import concourse.bass as bass
import concourse.bacc as bacc
import concourse.tile as tile
import concourse.mybir as mybir
import concourse.bass_utils as bass_utils

N_CORES = 8
B, C, L = 32, 32, 511
B_LOC = B // N_CORES  # 4
EPS = 1e-5
SCALE = float(C) ** -0.5

F32 = mybir.dt.float32
CONV_DT = getattr(mybir.dt, os.environ.get("KERNEL_CONV_DT", "float16"))
ATTN_DT = getattr(mybir.dt, os.environ.get("KERNEL_ATTN_DT", "float16"))

M_CHUNKS = [(0, 128), (128, 128), (256, 128), (384, 127)]  # m-chunks of L=511
# l-chunks of the gathered tensor: chunk X covers conv w0 range, sourcing
# l in [l0, l0+259). 259 = 256 + 3 so that t+g never leaves the chunk
# (reads for l >= 511 land in the zero tail of u4).
CHUNK_A = dict(l0=0, w0s=range(0, 64))
CHUNK_B = dict(l0=256, w0s=range(64, 128))
CW = 259


def _build_nc():
    nc = bacc.Bacc("TRN2", target_bir_lowering=False, debug=False,
                   num_devices=N_CORES)

    x_in = nc.dram_tensor("x", [B_LOC, C, L], F32, kind="ExternalInput")
    nw_in = nc.dram_tensor("norm_w", [C], F32, kind="ExternalInput")
    nb_in = nc.dram_tensor("norm_b", [C], F32, kind="ExternalInput")
    qkvw_in = nc.dram_tensor("qkv_w", [3 * C, C], F32, kind="ExternalInput")
    qkvb_in = nc.dram_tensor("qkv_b", [3 * C], F32, kind="ExternalInput")
    pw_in = nc.dram_tensor("proj_w", [C, C], F32, kind="ExternalInput")
    pb_in = nc.dram_tensor("proj_b", [C], F32, kind="ExternalInput")
    y_out = nc.dram_tensor("y", [B_LOC, C, L], F32, kind="ExternalOutput")

    with tile.TileContext(nc) as tc, \
         nc.allow_low_precision(reason="fp16 matmul operands; psum stays fp32"):
        with tc.tile_pool(name="const", bufs=1) as cpool, \
             tc.tile_pool(name="work", bufs=2) as wpool, \
             tc.tile_pool(name="esb", bufs=16) as epool, \
             tc.tile_pool(name="big", bufs=1) as bpool, \
             tc.tile_pool(name="pqk", bufs=1, space="PSUM") as pqk, \
             tc.tile_pool(name="psT", bufs=2, space="PSUM") as psT, \
             tc.tile_pool(name="pvt", bufs=1, space="PSUM") as pvt, \
             tc.tile_pool(name="pout", bufs=1, space="PSUM") as pout, \
             tc.tile_pool(name="dram", bufs=1, space="DRAM") as dram:

            # ---------------- weights / constants ----------------
            x_all = cpool.tile([128, L], F32, tag="x_all")
            nc.sync.dma_start(x_all[:], x_in.ap().rearrange("j c l -> (j c) l"))

            # qkv_w: contiguous load, per-32x32-block DVE transpose + cast,
            # then assemble q|k|v columns and replicate per batch row-block
            qw_raw = cpool.tile([3 * C, C], F32, tag="qw_raw")
            nc.sync.dma_start(qw_raw[:], qkvw_in.ap())
            qwT_blk = cpool.tile([3 * C, C], F32, tag="qwT_blk")
            nc.vector.transpose(qwT_blk[:], qw_raw[:])
            qwT16 = cpool.tile([3 * C, C], ATTN_DT, tag="qwT16")
            nc.vector.tensor_copy(qwT16[:], qwT_blk[:])
            wqkvT16 = cpool.tile([C, 3 * C], ATTN_DT, tag="wqkvT16")
            for k in range(3):
                nc.sync.dma_start(wqkvT16[:, 32 * k:32 * k + 32],
                                  qwT16[32 * k:32 * k + 32, :])
            wqkv_rep = cpool.tile([128, 3 * C], ATTN_DT, tag="wqkv_rep")
            for b_ in range(B_LOC):
                sl = slice(32 * b_, 32 * b_ + 32)
                nc.sync.dma_start(wqkv_rep[sl, :], wqkvT16[:])
            wq_rep = wqkv_rep[:, 0:32]
            wk_rep = wqkv_rep[:, 32:64]
            wv_rep = wqkv_rep[:, 64:96]

            pw_raw = cpool.tile([C, C], F32, tag="pw_raw")
            nc.sync.dma_start(pw_raw[:], pw_in.ap())
            pwT_blk = cpool.tile([C, C], F32, tag="pwT_blk")
            nc.vector.transpose(pwT_blk[:], pw_raw[:])
            wp_t16 = cpool.tile([C, C], CONV_DT, tag="wp_t16")
            nc.vector.tensor_copy(wp_t16[:], pwT_blk[:])
            wp_t = wp_t16[:]

            nw_rep = cpool.tile([128, 1], F32, tag="nw")
            nb_rep = cpool.tile([128, 1], F32, tag="nb")
            pb_rep = cpool.tile([128, 1], F32, tag="pb")
            qkb_rep = cpool.tile([128, 2], F32, tag="qkb")
            qkb_src = qkvb_in.ap()[0:64].rearrange("(j c) -> c j", j=2)
            for b_ in range(B_LOC):
                sl = slice(32 * b_, 32 * b_ + 32)
                nc.gpsimd.dma_start(nw_rep[sl, :],
                                    nw_in.ap().rearrange("(c o) -> c o", o=1))
                nc.gpsimd.dma_start(nb_rep[sl, :],
                                    nb_in.ap().rearrange("(c o) -> c o", o=1))
            for b_ in range(B_LOC):
                sl = slice(32 * b_, 32 * b_ + 32)
                nc.gpsimd.dma_start(pb_rep[sl, :],
                                    pb_in.ap().rearrange("(c o) -> c o", o=1))
                nc.gpsimd.dma_start(qkb_rep[sl, :], qkb_src)
            qb_rep = qkb_rep[:, 0:1]
            kb_rep = qkb_rep[:, 1:2]

            # v-bias broadcast [128, 32] via K=1 ones matmul (vb is on free dim)
            vb_st = cpool.tile([1, C], F32, tag="vb_st")
            nc.gpsimd.dma_start(vb_st[:], qkvb_in.ap()
                                .rearrange("(p o) -> p o", p=1)[:, 64:96])
            vb_row = cpool.tile([1, C], ATTN_DT, tag="vb_row")
            nc.vector.tensor_copy(vb_row[:], vb_st[:])
            ones_r0 = cpool.tile([1, 128], ATTN_DT, tag="ones_r0")
            nc.vector.memset(ones_r0[:], 1.0)
            ps_vb = pvt.tile([128, C], F32, tag="ps_vt", name="ps_vb")
            nc.tensor.matmul(ps_vb[:], ones_r0[:], vb_row[:], start=True,
                             stop=True)
            vb_bcast = cpool.tile([128, C], F32, tag="vb_bcast")
            nc.scalar.copy(vb_bcast[:], ps_vb[:])

            bc_ones = cpool.tile([128, 32], ATTN_DT, tag="bc_ones")
            nc.vector.memset(bc_ones[:], 1.0)
            ones128 = cpool.tile([128, 1], ATTN_DT, tag="ones128")
            nc.vector.memset(ones128[:], 1.0)

            # ---------------- group norm (per-channel over L) ----------------
            bnst = wpool.tile([128, 6], F32, tag="bnst")
            bnag = wpool.tile([128, 2], F32, tag="bnag")
            nc.vector.bn_stats(bnst[:], x_all[:])
            nc.vector.bn_aggr(bnag[:], bnst[:])
            eps_t = cpool.tile([128, 1], F32, tag="eps")
            nc.gpsimd.memset(eps_t[:], EPS)
            std = wpool.tile([128, 1], F32, tag="std")
            nc.scalar.activation(std[:], bnag[:, 1:2],
                                 mybir.ActivationFunctionType.Sqrt,
                                 bias=eps_t[:, 0:1])
            rstd = wpool.tile([128, 1], F32, tag="rstd")
            nc.vector.reciprocal(rstd[:], std[:])
            alpha = wpool.tile([128, 1], F32, tag="alpha")
            beta = wpool.tile([128, 1], F32, tag="beta")
            nc.vector.tensor_mul(alpha[:], rstd[:], nw_rep[:])
            nc.vector.tensor_mul(beta[:], bnag[:, 0:1], alpha[:])
            nc.vector.tensor_sub(beta[:], nb_rep[:], beta[:])
            h_all = cpool.tile([128, L], ATTN_DT, tag="h_all")
            nc.vector.tensor_scalar(h_all[:], x_all[:], alpha[:, 0:1],
                                    beta[:, 0:1], mybir.AluOpType.mult,
                                    mybir.AluOpType.add)

            # ---------------- attention, 4 batches packed ----------------
            # q/k: diagonal 32x32 tiles (row group b -> col group b)
            ps_q4 = pqk.tile([128, L], F32, tag="ps_q4", padded_shape=[128, 512])
            ps_k4 = pqk.tile([128, L], F32, tag="ps_k4", padded_shape=[128, 512])
            for b_ in range(B_LOC):
                sl = slice(32 * b_, 32 * b_ + 32)
                nc.tensor.matmul(ps_q4[sl, :], wqkv_rep[sl, 0:32], h_all[sl, :],
                                 start=True, stop=True,
                                 tile_position=(32 * b_, 32 * b_))
                nc.tensor.matmul(ps_k4[sl, :], wqkv_rep[sl, 32:64], h_all[sl, :],
                                 start=True, stop=True,
                                 tile_position=(32 * b_, 32 * b_))
            q4_sb = cpool.tile([128, L], ATTN_DT, tag="q4_sb")
            k4_sb = cpool.tile([128, L], ATTN_DT, tag="k4_sb")
            nc.vector.tensor_scalar(q4_sb[:], ps_q4[:], qb_rep, None,
                                    mybir.AluOpType.add)
            nc.vector.tensor_scalar(k4_sb[:], ps_k4[:], kb_rep, None,
                                    mybir.AluOpType.add)

            # V^T chunks per batch: [m-chunk, 32] (+ v bias)
            vt_sb = {}
            for ci, (m0, mc) in enumerate(M_CHUNKS):
                for b_ in range(B_LOC):
                    sl = slice(32 * b_, 32 * b_ + 32)
                    ps_vt = pvt.tile([128, C], F32, tag="ps_vt",
                                     name=f"ps_vt{ci}_{b_}")
                    nc.tensor.matmul(ps_vt[0:mc, :], h_all[sl, m0:m0 + mc],
                                     wqkv_rep[sl, 64:96], start=True, stop=True,
                                     tile_position=(32 * b_, 0))
                    vt = epool.tile([128, C], ATTN_DT, tag="vt",
                                    name=f"vt{ci}_{b_}")
                    nc.vector.tensor_add(vt[0:mc, :], ps_vt[0:mc, :],
                                         vb_bcast[0:mc, :])
                    vt_sb[(ci, b_)] = vt

            # scores S^T = K^T Q chunks, exp via ScalarE (scale folded in)
            e_sb = {}
            for ci, (m0, mc) in enumerate(M_CHUNKS):
                for b_ in range(B_LOC):
                    sl = slice(32 * b_, 32 * b_ + 32)
                    ps_sT = psT.tile([128, L], F32, tag="ps_sT", padded_shape=[128, 512],
                                     name=f"ps_sT{ci}_{b_}")
                    nc.tensor.matmul(ps_sT[0:mc, :], k4_sb[sl, m0:m0 + mc],
                                     q4_sb[sl, :], start=True, stop=True,
                                     tile_position=(32 * b_, 0))
                    e_t = epool.tile([128, L], ATTN_DT, tag="e_t",
                                     name=f"e{ci}_{b_}")
                    nc.scalar.activation(e_t[0:mc, :], ps_sT[0:mc, :],
                                         mybir.ActivationFunctionType.Exp,
                                         scale=SCALE)
                    e_sb[(ci, b_)] = e_t

            # out = V @ E^T (col-group per batch) and sumexp (M=1 per batch),
            # split in two n-halves so the first AllGather launches while the
            # second half is still computing.
            N_HALVES = [(0, CW), (CW, L - CW)]  # [0,259), [259,511)
            ps_o4 = pout.tile([128, L], F32, tag="ps_o4", padded_shape=[128, 512])
            ps_se = pout.tile([128, L], F32, tag="ps_se", padded_shape=[128, 512])
            recip_sb = cpool.tile([128, L], ATTN_DT, tag="recip_sb")
            ps_bc4 = pvt.tile([128, L], F32, tag="ps_bc4", padded_shape=[128, 512])
            u_un4 = cpool.tile([128, L], F32, tag="u_un4")
            # u4 padded to 520: columns [511, 520) stay zero so shifted
            # conv-weight reads for l >= 511 see zeros.
            u4 = cpool.tile([128, 520], CONV_DT, tag="u4")
            nc.vector.memset(u4[:, 508:520], 0.0)

            cc_in, cc_out = {}, {}
            for key in ("A", "B"):
                cc_in[key] = dram.tile([C * B_LOC * CW], CONV_DT,
                                       name=f"cc_in_{key}")
                cc_out[key] = dram.tile([N_CORES * C * B_LOC * CW], CONV_DT,
                                        addr_space="Shared",
                                        name=f"cc_out_{key}")

            for hi, (n0, nw_) in enumerate(N_HALVES):
                ns = slice(n0, n0 + nw_)
                for ci, (m0, mc) in enumerate(M_CHUNKS):
                    st, sp = (ci == 0), (ci == len(M_CHUNKS) - 1)
                    for b_ in range(B_LOC):
                        sl = slice(32 * b_, 32 * b_ + 32)
                        nc.tensor.matmul(ps_o4[sl, ns],
                                         vt_sb[(ci, b_)][0:mc, :],
                                         e_sb[(ci, b_)][0:mc, ns],
                                         start=st, stop=sp,
                                         tile_position=(0, 32 * b_),
                                         skip_group_check=True)
                        nc.tensor.matmul(ps_se[32 * b_:32 * b_ + 1, ns],
                                         ones128[0:mc, :],
                                         e_sb[(ci, b_)][0:mc, ns],
                                         start=st, stop=sp,
                                         tile_position=(0, 32 * b_),
                                         skip_group_check=True)
                for b_ in range(B_LOC):
                    r0 = slice(32 * b_, 32 * b_ + 1)
                    sl = slice(32 * b_, 32 * b_ + 32)
                    nc.vector.reciprocal(recip_sb[r0, ns], ps_se[r0, ns])
                    nc.tensor.matmul(ps_bc4[sl, ns], bc_ones[r0, :],
                                     recip_sb[r0, ns], start=True, stop=True,
                                     tile_position=(32 * b_, 32 * b_),
                                     skip_group_check=True)
                nc.scalar.copy(u_un4[:, ns], ps_o4[:, ns])
                nc.vector.tensor_mul(u4[:, ns], u_un4[:, ns], ps_bc4[:, ns])

                # half A == gather chunk A; launch its AllGather right away
                if hi == 0:
                    dst3 = cc_in["A"][:].rearrange("(i o l) -> i o l", i=C,
                                                   o=B_LOC)
                    for b_ in range(B_LOC):
                        nc.sync.dma_start(dst3[:, b_:b_ + 1, :],
                                          u4[32 * b_:32 * b_ + 32, 0:CW])
                    nc.gpsimd.collective_compute(
                        "AllGather", mybir.AluOpType.bypass,
                        replica_groups=[list(range(N_CORES))],
                        ins=[cc_in["A"][:]], outs=[cc_out["A"][:]])

            dst3 = cc_in["B"][:].rearrange("(i o l) -> i o l", i=C, o=B_LOC)
            for b_ in range(B_LOC):
                nc.sync.dma_start(dst3[:, b_:b_ + 1, :],
                                  u4[32 * b_:32 * b_ + 32, CHUNK_B["l0"]:
                                     CHUNK_B["l0"] + CW])
            nc.gpsimd.collective_compute(
                "AllGather", mybir.AluOpType.bypass,
                replica_groups=[list(range(N_CORES))],
                ins=[cc_in["B"][:]], outs=[cc_out["B"][:]])

            # conv rhs tiles (shifted copies of local batches, zero-padded)
            r_tiles = []
            for j in range(B_LOC):
                r_t = bpool.tile([128, 1024], CONV_DT, tag=f"r{j}",
                                 name=f"r{j}")
                for g in range(4):
                    nc.vector.memset(r_t[32 * g:32 * g + 32, 0:255 - g], 0.0)
                    nc.vector.memset(
                        r_t[32 * g:32 * g + 32, 255 - g + L:1024], 0.0)
                    nc.sync.dma_start(
                        r_t[32 * g:32 * g + 32, 255 - g:255 - g + L],
                        u4[32 * j:32 * j + 32, 0:L])
                r_tiles.append(r_t)

            # conv weights: each g-block is the full [i, (r,o,l)] shard
            # stored at free offset (3-g), so ut4X[(g,i), CW*(4r+o') + t]
            # = U[4r+o', i, l0 + t - 3 + g]; the per-o spill of (3-g)
            # elements lands in the next block's t < 3 region, never read
            # (lhsT slices use t = 4*w0 - l0 + 3 >= 3).
            ut4 = {}
            for key in ("A", "B"):
                t_u = bpool.tile([128, B * CW + 3], CONV_DT, tag=f"ut4{key}",
                                 name=f"ut4{key}")
                srcg = cc_out[key][:].rearrange(
                    "(r i o l) -> i r o l", r=N_CORES, i=C, o=B_LOC)
                for g in range(4):
                    dst = t_u[32 * g:32 * g + 32,
                              (3 - g):(3 - g) + B * CW].rearrange(
                        "p (r o t) -> p r o t", r=N_CORES, o=B_LOC)
                    nc.sync.dma_start(dst, srcg)
                ut4[key] = t_u

            # ---------------- self-conv ----------------
            ps_conv = psT.tile([128, L], F32, tag="ps_sT", padded_shape=[128, 512], name="ps_conv")
            for key, ch in (("A", CHUNK_A), ("B", CHUNK_B)):
                u3 = ut4[key][:, 0:B * CW].rearrange(
                    "p (r o t) -> p r o t", r=N_CORES, o=B_LOC)
                for w0 in ch["w0s"]:
                    toff = 4 * w0 - ch["l0"] + 3
                    lhsT = u3[:, :, :, toff:toff + 1]
                    for j in range(B_LOC):
                        nc.tensor.matmul(
                            ps_conv[32 * j:32 * j + 32, :],
                            lhsT,
                            r_tiles[j][:, 4 * w0:4 * w0 + L],
                            start=(w0 == 0), stop=(w0 == 127),
                            tile_position=(0, 32 * j),
                            skip_group_check=True)

            # ---------------- proj + residual ----------------
            ps_p = psT.tile([128, L], F32, tag="ps_sT", padded_shape=[128, 512], name="ps_p")
            for j in range(B_LOC):
                ysb = wpool.tile([32, L], CONV_DT, tag="ysb", name=f"ysb{j}")
                nc.vector.tensor_copy(ysb[:], ps_conv[32 * j:32 * j + 32, :])
                nc.tensor.matmul(ps_p[32 * j:32 * j + 32, :], wp_t, ysb[:],
                                 start=True, stop=True,
                                 tile_position=(0, 32 * j))
            res = cpool.tile([128, L], F32, tag="res")
            nc.vector.tensor_add(res[:], ps_p[:], x_all[:])
            nc.vector.tensor_scalar_add(res[:], res[:], pb_rep[:, 0:1])
            nc.sync.dma_start(y_out.ap().rearrange("j c l -> (j c) l"), res[:])

    nc.compile()
    return nc


_NC_CACHE = None


def kernel(x, norm_w, norm_b, qkv_w, qkv_b, proj_w, proj_b):
    global _NC_CACHE
    if _NC_CACHE is None:
        _NC_CACHE = _build_nc()
    nc = _NC_CACHE

    x = np.ascontiguousarray(np.asarray(x, dtype=np.float32))
    in_maps = []
    for c in range(N_CORES):
        in_maps.append({
            "x": x[c * B_LOC:(c + 1) * B_LOC],
            "norm_w": np.asarray(norm_w, np.float32),
            "norm_b": np.asarray(norm_b, np.float32),
            "qkv_w": np.asarray(qkv_w, np.float32),
            "qkv_b": np.asarray(qkv_b, np.float32),
            "proj_w": np.asarray(proj_w, np.float32),
            "proj_b": np.asarray(proj_b, np.float32),
        })

    trace = bool(int(os.environ.get("KERNEL_TRACE", "0")))
    res = bass_utils.run_bass_kernel_spmd(
        nc, in_maps, core_ids=list(range(N_CORES)), trace=trace)
    if trace and res.exec_time_ns is not None:
        print(f"HW exec time: {res.exec_time_ns} ns")

    out = np.concatenate([res.results[c]["y"] for c in range(N_CORES)], axis=0)
    return out.astype(np.float32)
